# revision 13
# baseline (speedup 1.0000x reference)
"""Circle Loss (PML-style) on 8 Trainium2 NeuronCores via Bass/Tile.

Full inputs -> full scalar output.

Fast path (masks match setup_inputs(): pos one-hot at (i+N)%2N, neg =
~pos & ~eye): exploits symmetry of E = exp(256*max(D,-1/4)^2 - 16)
(E_ij = E_ji) to compute only forward distances d in [1, 4096] per row
("half circle"), halving the exp work:

  Each core owns 1024 contiguous rows (rotated column layout makes the
  program core-independent). Per 128-row tile it computes a 4224-wide
  band of D via fp8 DoubleRow matmul, q = min(max(D,-1/4), M)^2 with
  triangular kill-masks fused into the PSUM drain
  (scalar_tensor_tensor), E = exp(256q-16) on ACT with accum_out row
  sums, and per-column sums of E via ones-stationary matmuls
  accumulated in PSUM across tiles. Host combines: row total = own
  fwd sum + sum of column credits from all cores - corrections
  (masked cells contribute e^-16; pos pair counted twice), then
  ln, softplus, mean. Host work is O(N*d).

General path (arbitrary masks): original masked-logsumexp kernel.
"""

import sys

sys.path.insert(0, "/opt/trn_rl_repo")

import numpy as np

TWO_N = 8192
D_EMB = 256
N_CORES = 8
ROWS_PER_CORE = TWO_N // N_CORES  # 1024
RT = ROWS_PER_CORE // 128  # 8 row tiles per core
BAND = 4224                # forward band per tile (4096 + 128 boundary)
CHUNK = 1408               # PSUM chunk (3 per band)
MOV_W = 128 * (RT - 1) + BAND  # 5120 rotated columns of moving operand
NSLICE = MOV_W // 512      # 10 absolute 512-col colsum slices

# per-tile template for tiles 1..6: 'A' = chunk1 on ACT (Square from
# PSUM, unclamped) with Pool sq c0 + DVE sq c2; 'B' = chunk1 clamp on
# DVE with Pool sq c1+c2 + DVE sq c0. Tiles 0 and 7 are special-cased.
ROUTE1 = ('-', 'A', 'B', 'A', 'B', 'A', 'B', '-')

_RUN_KWARGS: dict = {}
_NC_CACHE: dict = {}


def _split_waits(nc, maxw=1):
    """walrus in this container accepts at most ~2 sem-waits per
    instruction (1 for ACTIVATE); split extras onto preceding NoOps."""
    import concourse.mybir as mybir

    n_new = 0
    for bb in nc.main_func.blocks:
        insts = bb.instructions
        i = 0
        while i < len(insts):
            ins = insts[i]
            si = ins.sync_info
            if si is not None and si.on_wait and len(si.on_wait) > maxw:
                waits = list(si.on_wait)
                ins.sync_info = mybir.SyncInfo(
                    on_wait=waits[:maxw], on_update=si.on_update
                )
                rest = waits[maxw:]
                pos = i
                while rest:
                    chunk, rest = rest[:maxw], rest[maxw:]
                    nop = mybir.InstNoOp(name=f"I-waitfix-{n_new}")
                    n_new += 1
                    nop.engine = ins.engine
                    nop.sync_info = mybir.SyncInfo(on_wait=chunk, on_update=[])
                    insts.insert(pos, nop)
                    pos += 1
                    i += 1
            i += 1
    return n_new


def _build_nc_fast():
    import concourse.bass as bass
    import concourse.tile as tile
    from concourse import mybir

    f32 = mybir.dt.float32
    f16 = mybir.dt.float16
    bf16 = mybir.dt.bfloat16
    f8 = mybir.dt.float8e4
    AF = mybir.ActivationFunctionType
    ALU = mybir.AluOpType
    PM = mybir.MatmulPerfMode

    nc = bass.Bass("TRN2", target_bir_lowering=False)

    def reg_const(val, dtype=f32):
        t = nc.alloc_sbuf_tensor(f"const-{dtype.name}-{val}", [128, 1], dtype)
        nc.gpsimd.memset(t.ap(), val)
        nc.const_aps.aps[(dtype, val)] = t.ap()

    reg_const(-16.0)
    nc.all_engine_barrier()

    e8d = nc.dram_tensor("e8", [128, 2, MOV_W], f8, kind="ExternalInput")
    mhd = nc.dram_tensor("mh", [128, CHUNK], f16, kind="ExternalInput")
    mtd = nc.dram_tensor("mt", [128, CHUNK], f16, kind="ExternalInput")
    seld = nc.dram_tensor("sel", [128, 10 * NSLICE], bf16, kind="ExternalInput")
    s_out = nc.dram_tensor("s", [128, RT], f32, kind="ExternalOutput")
    cs_out = nc.dram_tensor("cs", [NSLICE, 512], f32, kind="ExternalOutput")

    def klast(s):
        return min(RT - 1, (512 * s + 511) // 128)

    with tile.TileContext(nc) as tc:
        with tc.tile_pool(name="singles", bufs=1) as singles, \
             tc.tile_pool(name="tpool", bufs=2) as tpool, \
             tc.tile_pool(name="qpool", bufs=2) as qpool, \
             tc.tile_pool(name="epool", bufs=3) as epool, \
             tc.tile_pool(name="psD", bufs=2, space="PSUM") as psD, \
             tc.tile_pool(name="psCS", bufs=1, space="PSUM") as psCS:

            mh = singles.tile([128, CHUNK], f16, tag="mh")
            nc.gpsimd.dma_start(out=mh, in_=mhd[:, :])
            mt = singles.tile([128, CHUNK], f16, tag="mt")
            nc.gpsimd.dma_start(out=mt, in_=mtd[:, :])
            sel = singles.tile([128, 10 * NSLICE], bf16, tag="sel")
            nc.gpsimd.dma_start(out=sel, in_=seld[:, :])
            e8 = singles.tile([128, 2, MOV_W], f8, tag="e8")
            # first chunk covers tile-0 chunk-0 band + stationary; spread
            # across hwdge queues (sync/vector/scalar) for parallelism
            dma_engs = (nc.sync, nc.sync, nc.gpsimd, nc.gpsimd)
            bounds = (0, 1536, 2732, 3928, 5120)
            for ch in range(4):
                c0, c1 = bounds[ch], bounds[ch + 1]
                dma_engs[ch].dma_start(
                    out=e8[:, :, c0:c1], in_=e8d[:, :, c0:c1]
                )

            s_all = singles.tile([128, RT], f32, tag="s_all")
            sparts = singles.tile([128, 4], f32, tag="sparts")
            csp = psCS.tile([NSLICE, 512], f32, tag="csp")
            nc.vector.memset(csp, 0.0)

            def emit_cs(k, e_t, r0, r1):
                # colsums of E band-rel cols [r0, r1) of tile k
                for s in range(NSLICE):
                    lo = max(512 * s, 128 * k + r0)
                    hi = min(512 * s + 512, 128 * k + r1)
                    if lo >= hi:
                        continue
                    nc.tensor.matmul(
                        csp[:, lo - 512 * s:hi - 512 * s],
                        sel[:, 10 * s:10 * s + 10],
                        e_t[:, lo - 128 * k:hi - 128 * k],
                        start=False,
                        stop=(k == klast(s) and hi == min(512 * s + 512,
                                                          128 * k + BAND)),
                        skip_group_check=True,
                    )

            pending = None
            sqr = 0  # index into SQ_ROUTE

            for k in range(RT):
                st = e8[:, :, 128 * k:128 * k + 128]
                ps_c = []
                for c in range(3):
                    a0 = 128 * k + CHUNK * c
                    ps = psD.tile([128, CHUNK], f32, tag="ps")
                    ps_c.append(ps)
                    for off, w in ((0, 512), (512, 512), (1024, 384)):
                        nc.tensor.matmul(
                            ps[:, off:off + w],
                            st,
                            e8[:, :, a0 + off:a0 + off + w],
                            start=True, stop=True,
                            perf_mode=PM.DoubleRow,
                            skip_group_check=True,
                        )
                    if k == 0 and c == 0:
                        # ramp: drain chunk0 immediately so exp can start
                        t_t0 = tpool.tile([128, BAND], f16, tag="t")
                        q_t0 = qpool.tile([128, BAND], f16, tag="q")
                        nc.vector.scalar_tensor_tensor(
                            t_t0[:, 0:CHUNK], ps, -0.25, mh,
                            op0=ALU.max, op1=ALU.mult,
                        )
                        nc.vector.tensor_tensor(
                            out=q_t0[:, 0:CHUNK], in0=t_t0[:, 0:CHUNK],
                            in1=t_t0[:, 0:CHUNK], op=ALU.mult,
                        )
                    if k == 0 and c == 1:
                        nc.scalar.activation(
                            q_t0[:, CHUNK:2 * CHUNK], ps, AF.Square,
                            bias=0.0, scale=1.0,
                        )

                e_t = epool.tile([128, BAND], bf16, tag="E")
                if k == 0:
                    # per-chunk exps for a fast pipeline ramp
                    t_t, q_t = t_t0, q_t0
                    nc.scalar.activation(
                        e_t[:, 0:CHUNK], q_t[:, 0:CHUNK], AF.Exp,
                        bias=-16.0, scale=256.0, accum_out=sparts[:, 0:1],
                    )
                    nc.scalar.activation(
                        e_t[:, CHUNK:2 * CHUNK], q_t[:, CHUNK:2 * CHUNK],
                        AF.Exp, bias=-16.0, scale=256.0,
                        accum_out=sparts[:, 1:2],
                    )
                    nc.vector.scalar_tensor_tensor(
                        t_t[:, 2 * CHUNK:3 * CHUNK], ps_c[2], -0.25, mt,
                        op0=ALU.max, op1=ALU.mult,
                    )
                    nc.vector.tensor_tensor(
                        out=q_t[:, 2 * CHUNK:3 * CHUNK],
                        in0=t_t[:, 2 * CHUNK:3 * CHUNK],
                        in1=t_t[:, 2 * CHUNK:3 * CHUNK], op=ALU.mult,
                    )
                    nc.scalar.activation(
                        e_t[:, 2 * CHUNK:3 * CHUNK], q_t[:, 2 * CHUNK:3 * CHUNK],
                        AF.Exp, bias=-16.0, scale=256.0,
                        accum_out=sparts[:, 2:3],
                    )
                    nc.vector.reduce_sum(
                        s_all[:, 0:1], sparts[:, 0:3], axis=mybir.AxisListType.X
                    )

                    def make_pending(e_t=e_t):
                        def go():
                            emit_cs(0, e_t, 0, BAND)
                        return go
                    pending = make_pending()
                    continue

                t_t = tpool.tile([128, BAND], f16, tag="t")
                q_t = qpool.tile([128, BAND], f16, tag="q")
                route = ROUTE1[k] if k != RT - 1 else 'T'

                def sq(eng, c):
                    eng.tensor_tensor(
                        out=q_t[:, c * CHUNK:(c + 1) * CHUNK],
                        in0=t_t[:, c * CHUNK:(c + 1) * CHUNK],
                        in1=t_t[:, c * CHUNK:(c + 1) * CHUNK],
                        op=ALU.mult,
                    )

                nc.vector.scalar_tensor_tensor(
                    t_t[:, 0:CHUNK], ps_c[0], -0.25, mh,
                    op0=ALU.max, op1=ALU.mult,
                )
                if route in ('A', 'T'):
                    if route == 'A':
                        sq(nc.gpsimd, 0)
                    else:
                        sq(nc.vector, 0)
                    nc.scalar.activation(
                        q_t[:, CHUNK:2 * CHUNK], ps_c[1], AF.Square,
                        bias=0.0, scale=1.0,
                    )
                    nc.vector.scalar_tensor_tensor(
                        t_t[:, 2 * CHUNK:3 * CHUNK], ps_c[2], -0.25, mt,
                        op0=ALU.max, op1=ALU.mult,
                    )
                    sq(nc.vector, 2)
                else:  # 'B'
                    sq(nc.vector, 0)
                    nc.vector.tensor_scalar(
                        t_t[:, CHUNK:2 * CHUNK], ps_c[1], -0.25, None, ALU.max
                    )
                    sq(nc.gpsimd, 1)
                    nc.vector.scalar_tensor_tensor(
                        t_t[:, 2 * CHUNK:3 * CHUNK], ps_c[2], -0.25, mt,
                        op0=ALU.max, op1=ALU.mult,
                    )
                    sq(nc.gpsimd, 2)

                if pending is not None:
                    pending()
                e_t = epool.tile([128, BAND], bf16, tag="E")

                if k == RT - 1:
                    # tail: per-chunk exps so colsums/drains overlap
                    for c in range(3):
                        nc.scalar.activation(
                            e_t[:, c * CHUNK:(c + 1) * CHUNK],
                            q_t[:, c * CHUNK:(c + 1) * CHUNK],
                            AF.Exp, bias=-16.0, scale=256.0,
                            accum_out=sparts[:, c:c + 1],
                        )
                        emit_cs(k, e_t, c * CHUNK, (c + 1) * CHUNK)
                        if c == 1:
                            # slices 0-6 are final after cols < 896+2816
                            cs_sb = singles.tile([NSLICE, 512], f32,
                                                 tag="cs_sb")
                            nc.vector.tensor_copy(out=cs_sb[0:7, :],
                                                  in_=csp[0:7, :])
                            nc.sync.dma_start(out=cs_out[0:7, :],
                                              in_=cs_sb[0:7, :])
                    nc.vector.reduce_sum(
                        s_all[:, k:k + 1], sparts[:, 0:3],
                        axis=mybir.AxisListType.X,
                    )
                    nc.sync.dma_start(out=s_out[:, :], in_=s_all)
                    pending = None
                else:
                    def make_exp_cs(k=k, q_t=q_t, e_t=e_t):
                        def go():
                            nc.scalar.activation(
                                e_t, q_t, AF.Exp, bias=-16.0, scale=256.0,
                                accum_out=s_all[:, k:k + 1],
                            )
                            emit_cs(k, e_t, 0, BAND)
                        return go
                    pending = make_exp_cs()

            if pending is not None:
                pending()

            nc.vector.tensor_copy(out=cs_sb, in_=csp)
            nc.sync.dma_start(out=cs_out[:, :], in_=cs_sb)

    _split_waits(nc)
    return nc


def _is_structured(pos_mask, neg_mask):
    pos_mask = np.asarray(pos_mask)
    neg_mask = np.asarray(neg_mask)
    idx = np.arange(TWO_N)
    pcols = (idx + TWO_N // 2) % TWO_N
    if not pos_mask[idx, pcols].all():
        return False
    if int(pos_mask.sum()) != TWO_N:
        return False
    if neg_mask[idx, idx].any() or neg_mask[idx, pcols].any():
        return False
    if int(neg_mask.sum()) != TWO_N * (TWO_N - 2):
        return False
    return True


def _kernel_fast(embeddings):
    import ml_dtypes
    from concourse.bass_utils import run_bass_kernel_spmd

    if "nc_fast" not in _NC_CACHE:
        _NC_CACHE["nc_fast"] = _build_nc_fast()
    nc = _NC_CACHE["nc_fast"]

    emb = np.asarray(embeddings, dtype=np.float32)
    e = emb / np.linalg.norm(emb.astype(np.float64), axis=1, keepdims=True)
    e = e.astype(np.float32)
    e8 = e.astype(ml_dtypes.float8_e4m3)
    e8f = e8.astype(np.float32)

    # moving operand: [128 part(K half), 2 halves, cols]
    e8T = np.ascontiguousarray(e8.T)  # [256, 8192]
    full_mov = e8T.reshape(2, 128, TWO_N).transpose(1, 0, 2)  # [128,2,8192]

    # fused triangular kill masks (x0 kills -> q=0 -> E=e^-16)
    tt, uu = np.meshgrid(np.arange(128), np.arange(128), indexing="ij")
    mh = np.full((128, CHUNK), 1.0, dtype=np.float16)
    mh[:, :128][uu <= tt] = 0.0
    mt = np.full((128, CHUNK), 1.0, dtype=np.float16)
    mt[:, 1280:][uu > tt] = 0.0

    sel = np.zeros((128, 10 * NSLICE), dtype=np.float32)
    for s in range(NSLICE):
        sel[:, 10 * s + s] = 1.0
    sel = sel.astype(ml_dtypes.bfloat16)

    in_maps = []
    for c in range(N_CORES):
        cols = (1024 * c + np.arange(MOV_W)) % TWO_N
        in_maps.append({
            "e8": np.ascontiguousarray(full_mov[:, :, cols]),
            "mh": mh, "mt": mt, "sel": sel,
        })

    res = run_bass_kernel_spmd(
        nc, in_maps, core_ids=list(range(N_CORES)), **_RUN_KWARGS
    )
    _NC_CACHE["last_result"] = res

    e16 = np.exp(np.float32(-16.0))
    e16b = np.float32(ml_dtypes.bfloat16(e16))

    R = np.empty(TWO_N, dtype=np.float64)
    C = np.zeros(TWO_N, dtype=np.float64)
    jj = np.arange(MOV_W)
    cnt = np.where(jj < 1024, 128 - jj % 128, 0) \
        + np.where(jj >= 4096, (jj - 4096) % 128, 0)
    ccorr = (cnt * e16b).astype(np.float64)
    for c in range(N_CORES):
        r = res.results[c]
        blk = np.asarray(r["s"], dtype=np.float64)  # [128, RT]
        rows = 1024 * c + 128 * np.arange(RT)[None, :] + np.arange(128)[:, None]
        R[rows.reshape(-1)] = blk.reshape(-1)
        cs = np.asarray(r["cs"], dtype=np.float64).reshape(-1)  # [5120]
        cols = (1024 * c + jj) % TWO_N
        np.add.at(C, cols, cs - ccorr)
    R -= 128.0 * e16

    idx = np.arange(TWO_N)
    pos = (idx + TWO_N // 2) % TWO_N
    d8 = np.einsum("ij,ij->i", e8f, e8f[pos]).astype(np.float32)
    tpos = np.maximum(d8, -0.25).astype(np.float16)
    qpos = (tpos * tpos).astype(np.float16)
    epos_f = np.exp(256.0 * qpos.astype(np.float32) - 16.0)
    epos_b = epos_f.astype(ml_dtypes.bfloat16).astype(np.float32)

    total = R + C - epos_f - epos_b
    lse_n = np.log(total)

    d_pos = np.einsum(
        "ij,ij->i", e.astype(np.float64), e[pos].astype(np.float64)
    )
    lse_p = 256.0 * ((d_pos - 1.0) ** 2 - 1.0 / 16.0)
    losses = np.logaddexp(0.0, lse_p + lse_n)
    return np.asarray(np.float32(losses.mean()))


def _build_nc(disjoint=False):
    import os
    import concourse.bass as bass
    import concourse.tile as tile
    from concourse import mybir

    no_inplace = os.environ.get("K_NOINPLACE", "0") == "1"
    no_gp = os.environ.get("K_NOGP", "0") == "1"
    no_exp = os.environ.get("K_NOEXP", "0") == "1"
    repeat = int(os.environ.get("K_REPEAT", "1"))

    CH = 2048
    NCH = TWO_N // CH
    B_SHIFT = 0.75
    ACT_ROUTE = (True, False, True, False)

    f32 = mybir.dt.float32
    f16 = mybir.dt.float16
    AF = mybir.ActivationFunctionType
    ALU = mybir.AluOpType

    nc = bass.Bass("TRN2", target_bir_lowering=False)

    def reg_const(val, dtype=f32):
        t = nc.alloc_sbuf_tensor(f"const-{dtype.name}-{val}", [128, 1], dtype)
        nc.gpsimd.memset(t.ap(), val)
        nc.const_aps.aps[(dtype, val)] = t.ap()

    for vv in (-1.0, 0.25, -0.25):
        reg_const(vv)
    nc.all_engine_barrier()

    eT = nc.dram_tensor("eT", [D_EMB, TWO_N], f16, kind="ExternalInput")
    erT = nc.dram_tensor("erT", [D_EMB, ROWS_PER_CORE], f16, kind="ExternalInput")
    posm = nc.dram_tensor("posm", [ROWS_PER_CORE, TWO_N], f16, kind="ExternalInput")
    negm = nc.dram_tensor("negm", [ROWS_PER_CORE, TWO_N], f16, kind="ExternalInput")
    loss_out = nc.dram_tensor("loss", [128, RT], f32, kind="ExternalOutput")

    with tile.TileContext(nc) as tc:
        with tc.tile_pool(name="singles", bufs=1) as singles, \
             tc.tile_pool(name="chunks", bufs=3) as chunks, \
             tc.tile_pool(name="masks", bufs=2) as maskp, \
             tc.tile_pool(name="arow", bufs=6) as arowp, \
             tc.tile_pool(name="rmax", bufs=2) as rmaxp, \
             tc.tile_pool(name="small", bufs=4) as small, \
             tc.tile_pool(name="tpool", bufs=3) as tpool, \
             tc.tile_pool(name="psum", bufs=2, space="PSUM") as psump:

            e_sb = []
            er_sb = []
            for k in range(2):
                t = singles.tile([128, TWO_N], f16, tag=f"e{k}")
                nc.sync.dma_start(out=t, in_=eT[k * 128:(k + 1) * 128, :])
                e_sb.append(t)
                tr = singles.tile([128, ROWS_PER_CORE], f16, tag=f"er{k}")
                nc.sync.dma_start(out=tr, in_=erT[k * 128:(k + 1) * 128, :])
                er_sb.append(tr)

            sp_all = singles.tile([128, RT], f32, tag="sp_all")
            sn_all = singles.tile([128, RT], f32, tag="sn_all")
            mp_all = singles.tile([128, RT], f32, tag="mp_all")
            mn_all = singles.tile([128, RT], f32, tag="mn_all")

            for rep in range(repeat):
              for rt in range(RT):
                r0 = rt * 128
                ap_cs = []
                an_cs = []
                rmp = rmaxp.tile([128, CH], f16, tag="rmp")
                rmn = rmaxp.tile([128, CH], f16, tag="rmn")
                for ch in range(NCH):
                    c0 = ch * CH
                    ps = psump.tile([128, CH], f32, tag="ps")
                    for sub in range(CH // 512):
                        s0 = sub * 512
                        for k in range(2):
                            nc.tensor.matmul(
                                ps[:, s0:s0 + 512],
                                er_sb[k][:, r0:r0 + 128],
                                e_sb[k][:, c0 + s0:c0 + s0 + 512],
                                start=(k == 0),
                                stop=(k == 1),
                            )
                    qp = chunks.tile([128, CH], f16, tag="qp")
                    nc.scalar.activation(qp, ps, AF.Square, bias=-1.0, scale=1.0)
                    v = chunks.tile([128, CH], f16, tag="v")
                    nc.scalar.activation(v, ps, AF.Relu, bias=0.25, scale=1.0)

                    pos_t = maskp.tile([128, CH], f16, tag="pos")
                    nc.sync.dma_start(out=pos_t, in_=posm[r0:r0 + 128, c0:c0 + CH])
                    neg_t = maskp.tile([128, CH], f16, tag="neg")
                    nc.sync.dma_start(out=neg_t, in_=negm[r0:r0 + 128, c0:c0 + CH])

                    ap_c = arowp.tile([128, CH], f16, tag="ap")
                    an_c = arowp.tile([128, CH], f16, tag="an")
                    ap_cs.append(ap_c)
                    an_cs.append(an_c)
                    if disjoint:
                        aB = chunks.tile([128, CH], f16, tag="aB")
                        nc.vector.tensor_scalar_add(aB, qp, B_SHIFT - 0.0625)
                        nc.vector.tensor_tensor(out=ap_c, in0=aB, in1=pos_t, op=ALU.mult)
                        bB = aB  # dead after ap_c
                        if ACT_ROUTE[ch]:
                            qvn = chunks.tile([128, CH], f16, tag="qvn")
                            nc.scalar.activation(qvn, v, AF.Square, bias=-0.25, scale=1.0)
                            nc.vector.tensor_scalar_add(bB, qvn, B_SHIFT - 0.0625)
                        else:
                            t5 = chunks.tile([128, CH], f16, tag="t5")
                            nc.vector.tensor_scalar_add(t5, v, -0.5)
                            u5 = chunks.tile([128, CH], f16, tag="u5")
                            nc.vector.tensor_tensor(out=u5, in0=t5, in1=v, op=ALU.mult)
                            nc.vector.tensor_scalar_add(bB, u5, B_SHIFT)
                        if no_gp:
                            nc.vector.tensor_tensor(out=an_c, in0=bB, in1=neg_t, op=ALU.mult)
                        else:
                            nc.gpsimd.tensor_tensor(out=an_c, in0=bB, in1=neg_t, op=ALU.mult)
                    else:
                        a_t = chunks.tile([128, CH], f16, tag="a_t")
                        nc.vector.tensor_scalar_add(a_t, qp, -0.0625)
                        pp = chunks.tile([128, CH], f16, tag="pp")
                        nc.vector.tensor_tensor(out=pp, in0=a_t, in1=pos_t, op=ALU.mult)
                        fnm = qp  # reuse dead qp slot
                        qvn = chunks.tile([128, CH], f16, tag="qvn")
                        nc.scalar.activation(qvn, v, AF.Square, bias=-0.25, scale=1.0)
                        b_t = a_t  # dead after pp
                        nc.vector.tensor_scalar_add(b_t, qvn, -0.0625)
                        nc.vector.tensor_tensor(out=fnm, in0=b_t, in1=neg_t, op=ALU.mult)
                        s_t = a_t
                        nc.vector.tensor_tensor(out=s_t, in0=pp, in1=fnm, op=ALU.add)
                        nw = v  # reuse dead v slot
                        nc.vector.tensor_scalar_add(nw, s_t, B_SHIFT)
                        nc.vector.tensor_tensor(out=ap_c, in0=nw, in1=pos_t, op=ALU.mult)
                        if no_gp:
                            nc.vector.tensor_tensor(out=an_c, in0=nw, in1=neg_t, op=ALU.mult)
                        else:
                            nc.gpsimd.tensor_tensor(out=an_c, in0=nw, in1=neg_t, op=ALU.mult)
                    if ch == 0:
                        nc.vector.tensor_copy(out=rmp, in_=ap_c)
                        nc.vector.tensor_copy(out=rmn, in_=an_c)
                    else:
                        nc.vector.tensor_tensor(out=rmp, in0=rmp, in1=ap_c, op=ALU.max)
                        nc.vector.tensor_tensor(out=rmn, in0=rmn, in1=an_c, op=ALU.max)

                tail_prio = tc.high_priority(offset=-70)
                tail_prio.__enter__()
                mp = mp_all[:, rt:rt + 1]
                nc.vector.reduce_max(mp, rmp[:, :], axis=mybir.AxisListType.X)
                mn = mn_all[:, rt:rt + 1]
                nc.vector.reduce_max(mn, rmn[:, :], axis=mybir.AxisListType.X)
                bias_p = small.tile([128, 1], f32, tag="bias_p")
                nc.vector.tensor_scalar_mul(bias_p, mp, -256.0)
                bias_n = small.tile([128, 1], f32, tag="bias_n")
                nc.vector.tensor_scalar_mul(bias_n, mn, -256.0)
                sp_parts = small.tile([128, NCH], f32, tag="sp_parts")
                sn_parts = small.tile([128, NCH], f32, tag="sn_parts")
                for ch in range(NCH if not no_exp else 0):
                    nc.scalar.activation(
                        ap_cs[ch], ap_cs[ch], AF.Exp, bias=bias_p[:, :], scale=256.0,
                        accum_out=sp_parts[:, ch:ch + 1],
                    )
                    nc.scalar.activation(
                        an_cs[ch], an_cs[ch], AF.Exp, bias=bias_n[:, :], scale=256.0,
                        accum_out=sn_parts[:, ch:ch + 1],
                    )
                if not no_exp:
                    nc.vector.reduce_sum(
                        sp_all[:, rt:rt + 1], sp_parts[:, :], axis=mybir.AxisListType.X
                    )
                    nc.vector.reduce_sum(
                        sn_all[:, rt:rt + 1], sn_parts[:, :], axis=mybir.AxisListType.X
                    )
                    tail_prio.__exit__(None, None, None)
                else:
                    nc.vector.tensor_copy(out=sp_all[:, rt:rt + 1], in_=bias_p)
                    nc.vector.tensor_copy(out=sn_all[:, rt:rt + 1], in_=bias_n)
                    tail_prio.__exit__(None, None, None)

            lp = small.tile([128, RT], f32, tag="lp")
            nc.scalar.activation(lp, sp_all, AF.Ln, bias=0.0, scale=1.0)
            ln_ = small.tile([128, RT], f32, tag="ln")
            nc.scalar.activation(ln_, sn_all, AF.Ln, bias=0.0, scale=1.0)
            msum = small.tile([128, RT], f32, tag="msum")
            nc.vector.tensor_tensor(out=msum, in0=mp_all, in1=mn_all, op=ALU.add)
            m256 = small.tile([128, RT], f32, tag="m256")
            nc.vector.tensor_scalar(
                m256, msum, -2.0 * B_SHIFT, 256.0, ALU.add, ALU.mult
            )
            lsum = small.tile([128, RT], f32, tag="lsum")
            nc.vector.tensor_tensor(out=lsum, in0=lp, in1=ln_, op=ALU.add)
            lse = small.tile([128, RT], f32, tag="lse")
            nc.vector.tensor_tensor(out=lse, in0=m256, in1=lsum, op=ALU.add)
            ax = small.tile([128, RT], f32, tag="ax")
            nc.scalar.activation(ax, lse, AF.Abs, bias=0.0, scale=1.0)
            et = small.tile([128, RT], f32, tag="et")
            nc.scalar.activation(et, ax, AF.Exp, bias=0.0, scale=-1.0)
            l1p = small.tile([128, RT], f32, tag="l1p")
            nc.scalar.activation(l1p, et, AF.Ln, bias=1.0, scale=1.0)
            rx = small.tile([128, RT], f32, tag="rx")
            nc.vector.tensor_scalar(rx, lse, 0.0, None, ALU.max)
            loss_t = small.tile([128, RT], f32, tag="loss")
            nc.vector.tensor_tensor(out=loss_t, in0=rx, in1=l1p, op=ALU.add)
            nc.sync.dma_start(out=loss_out[:, :], in_=loss_t)

    _split_waits(nc)
    return nc


def _kernel_general(embeddings, pos_mask, neg_mask):
    from concourse.bass_utils import run_bass_kernel_spmd

    disjoint = not bool(np.any(np.logical_and(np.asarray(pos_mask), np.asarray(neg_mask))))
    key = "nc_disjoint" if disjoint else "nc_general"
    if key not in _NC_CACHE:
        _NC_CACHE[key] = _build_nc(disjoint=disjoint)
    nc = _NC_CACHE[key]

    emb = np.asarray(embeddings, dtype=np.float32)
    e = emb / np.linalg.norm(emb.astype(np.float64), axis=1, keepdims=True)
    eT = np.ascontiguousarray(e.T).astype(np.float16)

    pos_f16 = np.asarray(pos_mask).astype(np.float16)
    neg_f16 = np.asarray(neg_mask).astype(np.float16)

    in_maps = []
    for c in range(N_CORES):
        r0 = c * ROWS_PER_CORE
        in_maps.append({
            "eT": eT,
            "erT": np.ascontiguousarray(eT[:, r0:r0 + ROWS_PER_CORE]),
            "posm": np.ascontiguousarray(pos_f16[r0:r0 + ROWS_PER_CORE]),
            "negm": np.ascontiguousarray(neg_f16[r0:r0 + ROWS_PER_CORE]),
        })

    res = run_bass_kernel_spmd(
        nc, in_maps, core_ids=list(range(N_CORES)), **_RUN_KWARGS
    )
    _NC_CACHE["last_result"] = res

    losses = np.empty(TWO_N, dtype=np.float32)
    for c in range(N_CORES):
        blk = res.results[c]["loss"]  # [128, RT]
        losses[c * ROWS_PER_CORE:(c + 1) * ROWS_PER_CORE] = blk.T.reshape(-1)

    valid = np.asarray(pos_mask).any(axis=1) & np.asarray(neg_mask).any(axis=1)
    losses = losses * valid.astype(np.float32)
    nz = losses > 0
    cnt = int(nz.sum())
    if cnt == 0:
        return np.zeros((), dtype=np.float32)
    mean = np.float32(losses.sum(dtype=np.float32) / np.float32(max(cnt, 1)))
    return np.asarray(mean, dtype=np.float32)


def kernel(embeddings: np.ndarray, pos_mask: np.ndarray, neg_mask: np.ndarray) -> np.ndarray:
    if _is_structured(pos_mask, neg_mask):
        return _kernel_fast(embeddings)
    return _kernel_general(embeddings, pos_mask, neg_mask)


# revision 14
# speedup vs baseline: 1.1114x; 1.1114x over previous
"""Circle Loss (PML-style) on 8 Trainium2 NeuronCores via Bass/Tile.

Full inputs -> full scalar output.

Fast path (masks match setup_inputs(): pos one-hot at (i+N)%2N, neg =
~pos & ~eye): exploits symmetry of E = exp(256*max(D,-1/4)^2 - 16)
(E_ij = E_ji) to compute only forward distances d in [1, 4096] per row
("half circle"), halving the exp work:

  Each core owns 1024 contiguous rows (rotated column layout makes the
  program core-independent). Per 128-row tile it computes a 4224-wide
  band of D via fp8 DoubleRow matmul, q = min(max(D,-1/4), M)^2 with
  triangular kill-masks fused into the PSUM drain
  (scalar_tensor_tensor), E = exp(256q-16) on ACT with accum_out row
  sums, and per-column sums of E via ones-stationary matmuls
  accumulated in PSUM across tiles. Host combines: row total = own
  fwd sum + sum of column credits from all cores - corrections
  (masked cells contribute e^-16; pos pair counted twice), then
  ln, softplus, mean. Host work is O(N*d).

General path (arbitrary masks): original masked-logsumexp kernel.
"""

import sys

sys.path.insert(0, "/opt/trn_rl_repo")

import numpy as np

TWO_N = 8192
D_EMB = 256
N_CORES = 8
ROWS_PER_CORE = TWO_N // N_CORES  # 1024
RT = ROWS_PER_CORE // 128  # 8 row tiles per core
BAND = 4224                # forward band per tile (4096 + 128 boundary)
CHUNK = 1408               # PSUM chunk (3 per band)
MOV_W = 128 * (RT - 1) + BAND  # 5120 rotated columns of moving operand
NSLICE = MOV_W // 512      # 10 absolute 512-col colsum slices

# per-tile template for tiles 1..6: 'A' = chunk1 on ACT (Square from
# PSUM, unclamped) with Pool sq c0 + DVE sq c2; 'B' = chunk1 clamp on
# DVE with Pool sq c1+c2 + DVE sq c0. Tiles 0 and 7 are special-cased.
ROUTE1 = ('-', 'A', 'A', 'A', 'A', 'A', 'A', '-')

_RUN_KWARGS: dict = {}
_NC_CACHE: dict = {}


def _split_waits(nc, maxw=1):
    """walrus in this container accepts at most ~2 sem-waits per
    instruction (1 for ACTIVATE); split extras onto preceding NoOps."""
    import concourse.mybir as mybir

    n_new = 0
    for bb in nc.main_func.blocks:
        insts = bb.instructions
        i = 0
        while i < len(insts):
            ins = insts[i]
            si = ins.sync_info
            if si is not None and si.on_wait and len(si.on_wait) > maxw:
                waits = list(si.on_wait)
                ins.sync_info = mybir.SyncInfo(
                    on_wait=waits[:maxw], on_update=si.on_update
                )
                rest = waits[maxw:]
                pos = i
                while rest:
                    chunk, rest = rest[:maxw], rest[maxw:]
                    nop = mybir.InstNoOp(name=f"I-waitfix-{n_new}")
                    n_new += 1
                    nop.engine = ins.engine
                    nop.sync_info = mybir.SyncInfo(on_wait=chunk, on_update=[])
                    insts.insert(pos, nop)
                    pos += 1
                    i += 1
            i += 1
    return n_new


def _build_nc_fast():
    import concourse.bass as bass
    import concourse.tile as tile
    from concourse import mybir

    f32 = mybir.dt.float32
    f16 = mybir.dt.float16
    bf16 = mybir.dt.bfloat16
    f8 = mybir.dt.float8e4
    AF = mybir.ActivationFunctionType
    ALU = mybir.AluOpType
    PM = mybir.MatmulPerfMode

    nc = bass.Bass("TRN2", target_bir_lowering=False)

    def reg_const(val, dtype=f32):
        t = nc.alloc_sbuf_tensor(f"const-{dtype.name}-{val}", [128, 1], dtype)
        nc.gpsimd.memset(t.ap(), val)
        nc.const_aps.aps[(dtype, val)] = t.ap()

    reg_const(-16.0)
    nc.all_engine_barrier()

    e8d = nc.dram_tensor("e8", [128, 2, MOV_W], f8, kind="ExternalInput")
    mhd = nc.dram_tensor("mh", [128, CHUNK], f16, kind="ExternalInput")
    mtd = nc.dram_tensor("mt", [128, CHUNK], f16, kind="ExternalInput")
    seld = nc.dram_tensor("sel", [128, 10 * NSLICE], bf16, kind="ExternalInput")
    s_out = nc.dram_tensor("s", [128, RT], f32, kind="ExternalOutput")
    cs_out = nc.dram_tensor("cs", [NSLICE, 512], f32, kind="ExternalOutput")

    def klast(s):
        return min(RT - 1, (512 * s + 511) // 128)

    with tile.TileContext(nc) as tc:
        with tc.tile_pool(name="singles", bufs=1) as singles, \
             tc.tile_pool(name="tpool", bufs=2) as tpool, \
             tc.tile_pool(name="qpool", bufs=2) as qpool, \
             tc.tile_pool(name="epool", bufs=3) as epool, \
             tc.tile_pool(name="psD", bufs=2, space="PSUM") as psD, \
             tc.tile_pool(name="psCS", bufs=1, space="PSUM") as psCS:

            mh = singles.tile([128, CHUNK], f16, tag="mh")
            nc.gpsimd.dma_start(out=mh, in_=mhd[:, :])
            mt = singles.tile([128, CHUNK], f16, tag="mt")
            nc.gpsimd.dma_start(out=mt, in_=mtd[:, :])
            sel = singles.tile([128, 10 * NSLICE], bf16, tag="sel")
            nc.gpsimd.dma_start(out=sel, in_=seld[:, :])
            e8 = singles.tile([128, 2, MOV_W], f8, tag="e8")
            # first chunk covers tile-0 chunk-0 band + stationary; spread
            # across hwdge queues (sync/vector/scalar) for parallelism
            dma_engs = (nc.sync, nc.sync, nc.gpsimd, nc.gpsimd)
            bounds = (0, 1536, 2732, 3928, 5120)
            for ch in range(4):
                c0, c1 = bounds[ch], bounds[ch + 1]
                dma_engs[ch].dma_start(
                    out=e8[:, :, c0:c1], in_=e8d[:, :, c0:c1]
                )

            s_all = singles.tile([128, RT], f32, tag="s_all")
            sparts = singles.tile([128, 4], f32, tag="sparts")
            csp = psCS.tile([NSLICE, 512], f32, tag="csp")
            nc.vector.memset(csp, 0.0)

            def emit_cs(k, e_t, r0, r1):
                # colsums of E band-rel cols [r0, r1) of tile k
                for s in range(NSLICE):
                    lo = max(512 * s, 128 * k + r0)
                    hi = min(512 * s + 512, 128 * k + r1)
                    if lo >= hi:
                        continue
                    nc.tensor.matmul(
                        csp[:, lo - 512 * s:hi - 512 * s],
                        sel[:, 10 * s:10 * s + 10],
                        e_t[:, lo - 128 * k:hi - 128 * k],
                        start=False,
                        stop=(k == klast(s) and hi == min(512 * s + 512,
                                                          128 * k + BAND)),
                        skip_group_check=True,
                    )

            pending = None
            sqr = 0  # index into SQ_ROUTE

            for k in range(RT):
                st = e8[:, :, 128 * k:128 * k + 128]
                ps_c = []
                for c in range(3):
                    a0 = 128 * k + CHUNK * c
                    ps = psD.tile([128, CHUNK], f32, tag="ps")
                    ps_c.append(ps)
                    for off, w in ((0, 512), (512, 512), (1024, 384)):
                        nc.tensor.matmul(
                            ps[:, off:off + w],
                            st,
                            e8[:, :, a0 + off:a0 + off + w],
                            start=True, stop=True,
                            perf_mode=PM.DoubleRow,
                            skip_group_check=True,
                        )
                    if k == 0 and c == 0:
                        # ramp: drain chunk0 immediately so exp can start
                        t_t0 = tpool.tile([128, BAND], f16, tag="t")
                        q_t0 = qpool.tile([128, BAND], f16, tag="q")
                        nc.vector.scalar_tensor_tensor(
                            t_t0[:, 0:CHUNK], ps, -0.25, mh,
                            op0=ALU.max, op1=ALU.mult,
                        )
                        nc.vector.tensor_tensor(
                            out=q_t0[:, 0:CHUNK], in0=t_t0[:, 0:CHUNK],
                            in1=t_t0[:, 0:CHUNK], op=ALU.mult,
                        )
                    if k == 0 and c == 1:
                        nc.scalar.activation(
                            q_t0[:, CHUNK:2 * CHUNK], ps, AF.Square,
                            bias=0.0, scale=1.0,
                        )

                e_t = epool.tile([128, BAND], bf16, tag="E")
                if k == 0:
                    # per-chunk exps for a fast pipeline ramp
                    t_t, q_t = t_t0, q_t0
                    nc.scalar.activation(
                        e_t[:, 0:CHUNK], q_t[:, 0:CHUNK], AF.Exp,
                        bias=-16.0, scale=256.0, accum_out=sparts[:, 0:1],
                    )
                    nc.scalar.activation(
                        e_t[:, CHUNK:2 * CHUNK], q_t[:, CHUNK:2 * CHUNK],
                        AF.Exp, bias=-16.0, scale=256.0,
                        accum_out=sparts[:, 1:2],
                    )
                    nc.vector.scalar_tensor_tensor(
                        t_t[:, 2 * CHUNK:3 * CHUNK], ps_c[2], -0.25, mt,
                        op0=ALU.max, op1=ALU.mult,
                    )
                    nc.vector.tensor_tensor(
                        out=q_t[:, 2 * CHUNK:3 * CHUNK],
                        in0=t_t[:, 2 * CHUNK:3 * CHUNK],
                        in1=t_t[:, 2 * CHUNK:3 * CHUNK], op=ALU.mult,
                    )
                    nc.scalar.activation(
                        e_t[:, 2 * CHUNK:3 * CHUNK], q_t[:, 2 * CHUNK:3 * CHUNK],
                        AF.Exp, bias=-16.0, scale=256.0,
                        accum_out=sparts[:, 2:3],
                    )
                    nc.vector.reduce_sum(
                        s_all[:, 0:1], sparts[:, 0:3], axis=mybir.AxisListType.X
                    )

                    def make_pending(e_t=e_t):
                        def go():
                            emit_cs(0, e_t, 0, BAND)
                        return go
                    pending = make_pending()
                    continue

                t_t = tpool.tile([128, BAND], f16, tag="t")
                q_t = qpool.tile([128, BAND], f16, tag="q")
                route = ROUTE1[k] if k != RT - 1 else 'T'

                def sq(eng, c):
                    eng.tensor_tensor(
                        out=q_t[:, c * CHUNK:(c + 1) * CHUNK],
                        in0=t_t[:, c * CHUNK:(c + 1) * CHUNK],
                        in1=t_t[:, c * CHUNK:(c + 1) * CHUNK],
                        op=ALU.mult,
                    )

                nc.vector.scalar_tensor_tensor(
                    t_t[:, 0:CHUNK], ps_c[0], -0.25, mh,
                    op0=ALU.max, op1=ALU.mult,
                )
                if route in ('A', 'T'):
                    if route == 'A':
                        sq(nc.gpsimd, 0)
                    else:
                        sq(nc.vector, 0)
                    nc.scalar.activation(
                        q_t[:, CHUNK:2 * CHUNK], ps_c[1], AF.Square,
                        bias=0.0, scale=1.0,
                    )
                    nc.vector.scalar_tensor_tensor(
                        t_t[:, 2 * CHUNK:3 * CHUNK], ps_c[2], -0.25, mt,
                        op0=ALU.max, op1=ALU.mult,
                    )
                    sq(nc.vector, 2)
                else:  # 'B'
                    sq(nc.vector, 0)
                    nc.vector.tensor_scalar(
                        t_t[:, CHUNK:2 * CHUNK], ps_c[1], -0.25, None, ALU.max
                    )
                    sq(nc.gpsimd, 1)
                    nc.vector.scalar_tensor_tensor(
                        t_t[:, 2 * CHUNK:3 * CHUNK], ps_c[2], -0.25, mt,
                        op0=ALU.max, op1=ALU.mult,
                    )
                    sq(nc.gpsimd, 2)

                if pending is not None:
                    pending()
                e_t = epool.tile([128, BAND], bf16, tag="E")

                if k == RT - 1:
                    # tail: per-chunk exps so colsums/drains overlap
                    for c in range(3):
                        nc.scalar.activation(
                            e_t[:, c * CHUNK:(c + 1) * CHUNK],
                            q_t[:, c * CHUNK:(c + 1) * CHUNK],
                            AF.Exp, bias=-16.0, scale=256.0,
                            accum_out=sparts[:, c:c + 1],
                        )
                        emit_cs(k, e_t, c * CHUNK, (c + 1) * CHUNK)
                        if c == 1:
                            # slices 0-6 are final after cols < 896+2816
                            cs_sb = singles.tile([NSLICE, 512], f32,
                                                 tag="cs_sb")
                            nc.vector.tensor_copy(out=cs_sb[0:7, :],
                                                  in_=csp[0:7, :])
                            nc.sync.dma_start(out=cs_out[0:7, :],
                                              in_=cs_sb[0:7, :])
                    nc.vector.reduce_sum(
                        s_all[:, k:k + 1], sparts[:, 0:3],
                        axis=mybir.AxisListType.X,
                    )
                    nc.sync.dma_start(out=s_out[:, :], in_=s_all)
                    pending = None
                else:
                    def make_exp_cs(k=k, q_t=q_t, e_t=e_t):
                        def go():
                            nc.scalar.activation(
                                e_t, q_t, AF.Exp, bias=-16.0, scale=256.0,
                                accum_out=s_all[:, k:k + 1],
                            )
                            emit_cs(k, e_t, 0, BAND)
                        return go
                    pending = make_exp_cs()

            if pending is not None:
                pending()

            nc.vector.tensor_copy(out=cs_sb, in_=csp)
            nc.sync.dma_start(out=cs_out[:, :], in_=cs_sb)

    _split_waits(nc)
    return nc


def _is_structured(pos_mask, neg_mask):
    pos_mask = np.asarray(pos_mask)
    neg_mask = np.asarray(neg_mask)
    idx = np.arange(TWO_N)
    pcols = (idx + TWO_N // 2) % TWO_N
    if not pos_mask[idx, pcols].all():
        return False
    if int(pos_mask.sum()) != TWO_N:
        return False
    if neg_mask[idx, idx].any() or neg_mask[idx, pcols].any():
        return False
    if int(neg_mask.sum()) != TWO_N * (TWO_N - 2):
        return False
    return True


def _kernel_fast(embeddings):
    import ml_dtypes
    from concourse.bass_utils import run_bass_kernel_spmd

    if "nc_fast" not in _NC_CACHE:
        _NC_CACHE["nc_fast"] = _build_nc_fast()
    nc = _NC_CACHE["nc_fast"]

    emb = np.asarray(embeddings, dtype=np.float32)
    e = emb / np.linalg.norm(emb.astype(np.float64), axis=1, keepdims=True)
    e = e.astype(np.float32)
    e8 = e.astype(ml_dtypes.float8_e4m3)
    e8f = e8.astype(np.float32)

    # moving operand: [128 part(K half), 2 halves, cols]
    e8T = np.ascontiguousarray(e8.T)  # [256, 8192]
    full_mov = e8T.reshape(2, 128, TWO_N).transpose(1, 0, 2)  # [128,2,8192]

    # fused triangular kill masks (x0 kills -> q=0 -> E=e^-16)
    tt, uu = np.meshgrid(np.arange(128), np.arange(128), indexing="ij")
    mh = np.full((128, CHUNK), 1.0, dtype=np.float16)
    mh[:, :128][uu <= tt] = 0.0
    mt = np.full((128, CHUNK), 1.0, dtype=np.float16)
    mt[:, 1280:][uu > tt] = 0.0

    sel = np.zeros((128, 10 * NSLICE), dtype=np.float32)
    for s in range(NSLICE):
        sel[:, 10 * s + s] = 1.0
    sel = sel.astype(ml_dtypes.bfloat16)

    in_maps = []
    for c in range(N_CORES):
        cols = (1024 * c + np.arange(MOV_W)) % TWO_N
        in_maps.append({
            "e8": np.ascontiguousarray(full_mov[:, :, cols]),
            "mh": mh, "mt": mt, "sel": sel,
        })

    res = run_bass_kernel_spmd(
        nc, in_maps, core_ids=list(range(N_CORES)), **_RUN_KWARGS
    )
    _NC_CACHE["last_result"] = res

    e16 = np.exp(np.float32(-16.0))
    e16b = np.float32(ml_dtypes.bfloat16(e16))

    R = np.empty(TWO_N, dtype=np.float64)
    C = np.zeros(TWO_N, dtype=np.float64)
    jj = np.arange(MOV_W)
    cnt = np.where(jj < 1024, 128 - jj % 128, 0) \
        + np.where(jj >= 4096, (jj - 4096) % 128, 0)
    ccorr = (cnt * e16b).astype(np.float64)
    for c in range(N_CORES):
        r = res.results[c]
        blk = np.asarray(r["s"], dtype=np.float64)  # [128, RT]
        rows = 1024 * c + 128 * np.arange(RT)[None, :] + np.arange(128)[:, None]
        R[rows.reshape(-1)] = blk.reshape(-1)
        cs = np.asarray(r["cs"], dtype=np.float64).reshape(-1)  # [5120]
        cols = (1024 * c + jj) % TWO_N
        np.add.at(C, cols, cs - ccorr)
    R -= 128.0 * e16

    idx = np.arange(TWO_N)
    pos = (idx + TWO_N // 2) % TWO_N
    d8 = np.einsum("ij,ij->i", e8f, e8f[pos]).astype(np.float32)
    tpos = np.maximum(d8, -0.25).astype(np.float16)
    qpos = (tpos * tpos).astype(np.float16)
    epos_f = np.exp(256.0 * qpos.astype(np.float32) - 16.0)
    epos_b = epos_f.astype(ml_dtypes.bfloat16).astype(np.float32)

    total = R + C - epos_f - epos_b
    lse_n = np.log(total)

    d_pos = np.einsum(
        "ij,ij->i", e.astype(np.float64), e[pos].astype(np.float64)
    )
    lse_p = 256.0 * ((d_pos - 1.0) ** 2 - 1.0 / 16.0)
    losses = np.logaddexp(0.0, lse_p + lse_n)
    return np.asarray(np.float32(losses.mean()))


def _build_nc(disjoint=False):
    import os
    import concourse.bass as bass
    import concourse.tile as tile
    from concourse import mybir

    no_inplace = os.environ.get("K_NOINPLACE", "0") == "1"
    no_gp = os.environ.get("K_NOGP", "0") == "1"
    no_exp = os.environ.get("K_NOEXP", "0") == "1"
    repeat = int(os.environ.get("K_REPEAT", "1"))

    CH = 2048
    NCH = TWO_N // CH
    B_SHIFT = 0.75
    ACT_ROUTE = (True, False, True, False)

    f32 = mybir.dt.float32
    f16 = mybir.dt.float16
    AF = mybir.ActivationFunctionType
    ALU = mybir.AluOpType

    nc = bass.Bass("TRN2", target_bir_lowering=False)

    def reg_const(val, dtype=f32):
        t = nc.alloc_sbuf_tensor(f"const-{dtype.name}-{val}", [128, 1], dtype)
        nc.gpsimd.memset(t.ap(), val)
        nc.const_aps.aps[(dtype, val)] = t.ap()

    for vv in (-1.0, 0.25, -0.25):
        reg_const(vv)
    nc.all_engine_barrier()

    eT = nc.dram_tensor("eT", [D_EMB, TWO_N], f16, kind="ExternalInput")
    erT = nc.dram_tensor("erT", [D_EMB, ROWS_PER_CORE], f16, kind="ExternalInput")
    posm = nc.dram_tensor("posm", [ROWS_PER_CORE, TWO_N], f16, kind="ExternalInput")
    negm = nc.dram_tensor("negm", [ROWS_PER_CORE, TWO_N], f16, kind="ExternalInput")
    loss_out = nc.dram_tensor("loss", [128, RT], f32, kind="ExternalOutput")

    with tile.TileContext(nc) as tc:
        with tc.tile_pool(name="singles", bufs=1) as singles, \
             tc.tile_pool(name="chunks", bufs=3) as chunks, \
             tc.tile_pool(name="masks", bufs=2) as maskp, \
             tc.tile_pool(name="arow", bufs=6) as arowp, \
             tc.tile_pool(name="rmax", bufs=2) as rmaxp, \
             tc.tile_pool(name="small", bufs=4) as small, \
             tc.tile_pool(name="tpool", bufs=3) as tpool, \
             tc.tile_pool(name="psum", bufs=2, space="PSUM") as psump:

            e_sb = []
            er_sb = []
            for k in range(2):
                t = singles.tile([128, TWO_N], f16, tag=f"e{k}")
                nc.sync.dma_start(out=t, in_=eT[k * 128:(k + 1) * 128, :])
                e_sb.append(t)
                tr = singles.tile([128, ROWS_PER_CORE], f16, tag=f"er{k}")
                nc.sync.dma_start(out=tr, in_=erT[k * 128:(k + 1) * 128, :])
                er_sb.append(tr)

            sp_all = singles.tile([128, RT], f32, tag="sp_all")
            sn_all = singles.tile([128, RT], f32, tag="sn_all")
            mp_all = singles.tile([128, RT], f32, tag="mp_all")
            mn_all = singles.tile([128, RT], f32, tag="mn_all")

            for rep in range(repeat):
              for rt in range(RT):
                r0 = rt * 128
                ap_cs = []
                an_cs = []
                rmp = rmaxp.tile([128, CH], f16, tag="rmp")
                rmn = rmaxp.tile([128, CH], f16, tag="rmn")
                for ch in range(NCH):
                    c0 = ch * CH
                    ps = psump.tile([128, CH], f32, tag="ps")
                    for sub in range(CH // 512):
                        s0 = sub * 512
                        for k in range(2):
                            nc.tensor.matmul(
                                ps[:, s0:s0 + 512],
                                er_sb[k][:, r0:r0 + 128],
                                e_sb[k][:, c0 + s0:c0 + s0 + 512],
                                start=(k == 0),
                                stop=(k == 1),
                            )
                    qp = chunks.tile([128, CH], f16, tag="qp")
                    nc.scalar.activation(qp, ps, AF.Square, bias=-1.0, scale=1.0)
                    v = chunks.tile([128, CH], f16, tag="v")
                    nc.scalar.activation(v, ps, AF.Relu, bias=0.25, scale=1.0)

                    pos_t = maskp.tile([128, CH], f16, tag="pos")
                    nc.sync.dma_start(out=pos_t, in_=posm[r0:r0 + 128, c0:c0 + CH])
                    neg_t = maskp.tile([128, CH], f16, tag="neg")
                    nc.sync.dma_start(out=neg_t, in_=negm[r0:r0 + 128, c0:c0 + CH])

                    ap_c = arowp.tile([128, CH], f16, tag="ap")
                    an_c = arowp.tile([128, CH], f16, tag="an")
                    ap_cs.append(ap_c)
                    an_cs.append(an_c)
                    if disjoint:
                        aB = chunks.tile([128, CH], f16, tag="aB")
                        nc.vector.tensor_scalar_add(aB, qp, B_SHIFT - 0.0625)
                        nc.vector.tensor_tensor(out=ap_c, in0=aB, in1=pos_t, op=ALU.mult)
                        bB = aB  # dead after ap_c
                        if ACT_ROUTE[ch]:
                            qvn = chunks.tile([128, CH], f16, tag="qvn")
                            nc.scalar.activation(qvn, v, AF.Square, bias=-0.25, scale=1.0)
                            nc.vector.tensor_scalar_add(bB, qvn, B_SHIFT - 0.0625)
                        else:
                            t5 = chunks.tile([128, CH], f16, tag="t5")
                            nc.vector.tensor_scalar_add(t5, v, -0.5)
                            u5 = chunks.tile([128, CH], f16, tag="u5")
                            nc.vector.tensor_tensor(out=u5, in0=t5, in1=v, op=ALU.mult)
                            nc.vector.tensor_scalar_add(bB, u5, B_SHIFT)
                        if no_gp:
                            nc.vector.tensor_tensor(out=an_c, in0=bB, in1=neg_t, op=ALU.mult)
                        else:
                            nc.gpsimd.tensor_tensor(out=an_c, in0=bB, in1=neg_t, op=ALU.mult)
                    else:
                        a_t = chunks.tile([128, CH], f16, tag="a_t")
                        nc.vector.tensor_scalar_add(a_t, qp, -0.0625)
                        pp = chunks.tile([128, CH], f16, tag="pp")
                        nc.vector.tensor_tensor(out=pp, in0=a_t, in1=pos_t, op=ALU.mult)
                        fnm = qp  # reuse dead qp slot
                        qvn = chunks.tile([128, CH], f16, tag="qvn")
                        nc.scalar.activation(qvn, v, AF.Square, bias=-0.25, scale=1.0)
                        b_t = a_t  # dead after pp
                        nc.vector.tensor_scalar_add(b_t, qvn, -0.0625)
                        nc.vector.tensor_tensor(out=fnm, in0=b_t, in1=neg_t, op=ALU.mult)
                        s_t = a_t
                        nc.vector.tensor_tensor(out=s_t, in0=pp, in1=fnm, op=ALU.add)
                        nw = v  # reuse dead v slot
                        nc.vector.tensor_scalar_add(nw, s_t, B_SHIFT)
                        nc.vector.tensor_tensor(out=ap_c, in0=nw, in1=pos_t, op=ALU.mult)
                        if no_gp:
                            nc.vector.tensor_tensor(out=an_c, in0=nw, in1=neg_t, op=ALU.mult)
                        else:
                            nc.gpsimd.tensor_tensor(out=an_c, in0=nw, in1=neg_t, op=ALU.mult)
                    if ch == 0:
                        nc.vector.tensor_copy(out=rmp, in_=ap_c)
                        nc.vector.tensor_copy(out=rmn, in_=an_c)
                    else:
                        nc.vector.tensor_tensor(out=rmp, in0=rmp, in1=ap_c, op=ALU.max)
                        nc.vector.tensor_tensor(out=rmn, in0=rmn, in1=an_c, op=ALU.max)

                tail_prio = tc.high_priority(offset=-70)
                tail_prio.__enter__()
                mp = mp_all[:, rt:rt + 1]
                nc.vector.reduce_max(mp, rmp[:, :], axis=mybir.AxisListType.X)
                mn = mn_all[:, rt:rt + 1]
                nc.vector.reduce_max(mn, rmn[:, :], axis=mybir.AxisListType.X)
                bias_p = small.tile([128, 1], f32, tag="bias_p")
                nc.vector.tensor_scalar_mul(bias_p, mp, -256.0)
                bias_n = small.tile([128, 1], f32, tag="bias_n")
                nc.vector.tensor_scalar_mul(bias_n, mn, -256.0)
                sp_parts = small.tile([128, NCH], f32, tag="sp_parts")
                sn_parts = small.tile([128, NCH], f32, tag="sn_parts")
                for ch in range(NCH if not no_exp else 0):
                    nc.scalar.activation(
                        ap_cs[ch], ap_cs[ch], AF.Exp, bias=bias_p[:, :], scale=256.0,
                        accum_out=sp_parts[:, ch:ch + 1],
                    )
                    nc.scalar.activation(
                        an_cs[ch], an_cs[ch], AF.Exp, bias=bias_n[:, :], scale=256.0,
                        accum_out=sn_parts[:, ch:ch + 1],
                    )
                if not no_exp:
                    nc.vector.reduce_sum(
                        sp_all[:, rt:rt + 1], sp_parts[:, :], axis=mybir.AxisListType.X
                    )
                    nc.vector.reduce_sum(
                        sn_all[:, rt:rt + 1], sn_parts[:, :], axis=mybir.AxisListType.X
                    )
                    tail_prio.__exit__(None, None, None)
                else:
                    nc.vector.tensor_copy(out=sp_all[:, rt:rt + 1], in_=bias_p)
                    nc.vector.tensor_copy(out=sn_all[:, rt:rt + 1], in_=bias_n)
                    tail_prio.__exit__(None, None, None)

            lp = small.tile([128, RT], f32, tag="lp")
            nc.scalar.activation(lp, sp_all, AF.Ln, bias=0.0, scale=1.0)
            ln_ = small.tile([128, RT], f32, tag="ln")
            nc.scalar.activation(ln_, sn_all, AF.Ln, bias=0.0, scale=1.0)
            msum = small.tile([128, RT], f32, tag="msum")
            nc.vector.tensor_tensor(out=msum, in0=mp_all, in1=mn_all, op=ALU.add)
            m256 = small.tile([128, RT], f32, tag="m256")
            nc.vector.tensor_scalar(
                m256, msum, -2.0 * B_SHIFT, 256.0, ALU.add, ALU.mult
            )
            lsum = small.tile([128, RT], f32, tag="lsum")
            nc.vector.tensor_tensor(out=lsum, in0=lp, in1=ln_, op=ALU.add)
            lse = small.tile([128, RT], f32, tag="lse")
            nc.vector.tensor_tensor(out=lse, in0=m256, in1=lsum, op=ALU.add)
            ax = small.tile([128, RT], f32, tag="ax")
            nc.scalar.activation(ax, lse, AF.Abs, bias=0.0, scale=1.0)
            et = small.tile([128, RT], f32, tag="et")
            nc.scalar.activation(et, ax, AF.Exp, bias=0.0, scale=-1.0)
            l1p = small.tile([128, RT], f32, tag="l1p")
            nc.scalar.activation(l1p, et, AF.Ln, bias=1.0, scale=1.0)
            rx = small.tile([128, RT], f32, tag="rx")
            nc.vector.tensor_scalar(rx, lse, 0.0, None, ALU.max)
            loss_t = small.tile([128, RT], f32, tag="loss")
            nc.vector.tensor_tensor(out=loss_t, in0=rx, in1=l1p, op=ALU.add)
            nc.sync.dma_start(out=loss_out[:, :], in_=loss_t)

    _split_waits(nc)
    return nc


def _kernel_general(embeddings, pos_mask, neg_mask):
    from concourse.bass_utils import run_bass_kernel_spmd

    disjoint = not bool(np.any(np.logical_and(np.asarray(pos_mask), np.asarray(neg_mask))))
    key = "nc_disjoint" if disjoint else "nc_general"
    if key not in _NC_CACHE:
        _NC_CACHE[key] = _build_nc(disjoint=disjoint)
    nc = _NC_CACHE[key]

    emb = np.asarray(embeddings, dtype=np.float32)
    e = emb / np.linalg.norm(emb.astype(np.float64), axis=1, keepdims=True)
    eT = np.ascontiguousarray(e.T).astype(np.float16)

    pos_f16 = np.asarray(pos_mask).astype(np.float16)
    neg_f16 = np.asarray(neg_mask).astype(np.float16)

    in_maps = []
    for c in range(N_CORES):
        r0 = c * ROWS_PER_CORE
        in_maps.append({
            "eT": eT,
            "erT": np.ascontiguousarray(eT[:, r0:r0 + ROWS_PER_CORE]),
            "posm": np.ascontiguousarray(pos_f16[r0:r0 + ROWS_PER_CORE]),
            "negm": np.ascontiguousarray(neg_f16[r0:r0 + ROWS_PER_CORE]),
        })

    res = run_bass_kernel_spmd(
        nc, in_maps, core_ids=list(range(N_CORES)), **_RUN_KWARGS
    )
    _NC_CACHE["last_result"] = res

    losses = np.empty(TWO_N, dtype=np.float32)
    for c in range(N_CORES):
        blk = res.results[c]["loss"]  # [128, RT]
        losses[c * ROWS_PER_CORE:(c + 1) * ROWS_PER_CORE] = blk.T.reshape(-1)

    valid = np.asarray(pos_mask).any(axis=1) & np.asarray(neg_mask).any(axis=1)
    losses = losses * valid.astype(np.float32)
    nz = losses > 0
    cnt = int(nz.sum())
    if cnt == 0:
        return np.zeros((), dtype=np.float32)
    mean = np.float32(losses.sum(dtype=np.float32) / np.float32(max(cnt, 1)))
    return np.asarray(mean, dtype=np.float32)


def kernel(embeddings: np.ndarray, pos_mask: np.ndarray, neg_mask: np.ndarray) -> np.ndarray:
    if _is_structured(pos_mask, neg_mask):
        return _kernel_fast(embeddings)
    return _kernel_general(embeddings, pos_mask, neg_mask)
